# revision 1
# baseline (speedup 1.0000x reference)
"""Contrastive-loss (InfoNCE / softmax-CE) kernel for 8 Trainium2 NeuronCores.

reference semantics:
    scores = feature @ anchor.T          # [B, CLS]
    loss   = mean_b( logsumexp(scores[b]) - scores[b, target[b]] )

Strategy (data-parallel, per sharding hint):
  - shard feature/_target along batch across 8 cores (2048 rows each),
    replicate anchor.
  - host prepares transposed layouts (featT [FEAT, BPC] per core, anchorT
    [FEAT, CLS]) so the contraction dim lands on SBUF partitions — pure
    layout prep in the sharding layer.
  - device: tiled matmul (PE) -> row-wise max (DVE) -> exp+sum (ACT, fused
    accumulate) -> target-score extraction via iota==target mask (DVE, fused
    accumulate) -> per-row nll -> DMA out [2048] per core.
  - host: mean over all 16384 rows (the scalar all-reduce).

Matmul dtype knob (CL_MM_DTYPE): f32 (exact, 4 cyc/row), f32r (fast fp32,
1 cyc/row), f32r2 (two-pass hi/lo split on f32r, ~fp32 precision, 2 cyc/row),
bf16 (1 cyc/row, lowest precision).
"""

import os
import sys
from contextlib import ExitStack

import numpy as np

for _p in ("/opt/trn_rl_repo",):
    if os.path.isdir(_p) and _p not in sys.path:
        sys.path.insert(0, _p)

import concourse.bass as bass
import concourse.bacc as bacc
import concourse.mybir as mybir
import concourse.tile as tile

B, CLS, FEAT = 16384, 1000, 2048
NCORES = 8
BPC = B // NCORES          # 2048 batch rows per core
P = 128                    # partitions
KT = FEAT // P             # 16 contraction tiles
MT = BPC // P              # 16 batch tiles per core
GRP = 4                    # m-tiles per feature-slab group (DMA batching)
NGRP = MT // GRP
N0 = 512                   # first class tile (one PSUM bank of fp32)
N1 = CLS - N0              # 488
NF = 2 * N0                # padded scores width (1024)
NEG_BIG = -3.0e38          # padding for unused score columns

MM_DTYPE = os.environ.get("CL_MM_DTYPE", "f32r")


def _mm_dt(mm_dtype: str):
    return {
        "f32": mybir.dt.float32,
        "f32r": mybir.dt.float32r,
        "f32r2": mybir.dt.float32r,
        "bf16": mybir.dt.bfloat16,
    }[mm_dtype]


def build_program(mm_dtype: str = MM_DTYPE, reps: int = 1,
                  loop_iters: int = 1) -> bass.Bass:
    """Build the per-core Bass/Tile program (SPMD: same program on all cores).

    reps > 1 repeats the full body (including all DMAs) for differential
    device-time measurement; loop_iters > 1 does the same with a hardware
    For_i loop (compact code, ~2us back-edge per iteration)."""
    f32 = mybir.dt.float32
    mdt = _mm_dt(mm_dtype)
    two_pass = mm_dtype == "f32r2"
    npass = 2 if two_pass else 1

    nc = bacc.Bacc(None, target_bir_lowering=False, debug=False)
    feat_shape = [npass * FEAT, BPC]
    featT = nc.dram_tensor("featT", feat_shape, mdt, kind="ExternalInput")
    anchorT = nc.dram_tensor("anchorT", [FEAT, CLS], mdt, kind="ExternalInput")
    tgt = nc.dram_tensor("tgt", [BPC], f32, kind="ExternalInput")
    nll = nc.dram_tensor("nll", [BPC], f32, kind="ExternalOutput")

    # [p, pass*kt, m] / [p, kt, c] views with the contraction dim on partitions
    fview = featT.ap().rearrange("(kt p) m -> p kt m", p=P)   # [128, npass*16, 2048]
    aview = anchorT.ap().rearrange("(kt p) c -> p kt c", p=P)  # [128, 16, 1000]

    with tile.TileContext(nc) as tc, ExitStack() as ctx:
        singles = ctx.enter_context(tc.tile_pool(name="singles", bufs=1))
        feats = ctx.enter_context(tc.tile_pool(name="feats", bufs=2))
        psum = ctx.enter_context(tc.tile_pool(name="psum", bufs=4, space="PSUM"))
        stats = ctx.enter_context(tc.tile_pool(name="stats", bufs=8))
        scratch = ctx.enter_context(tc.tile_pool(name="scratch", bufs=2))

        # iota row 0..CLS-1 (exact in f32), replicated on every partition
        iota_i = singles.tile([P, CLS], mybir.dt.int32)
        nc.gpsimd.iota(iota_i, pattern=[[1, CLS]], base=0, channel_multiplier=0)
        iota_f = singles.tile([P, CLS], f32)
        nc.vector.tensor_copy(out=iota_f, in_=iota_i)

        if loop_iters > 1:
            assert reps == 1
            with tc.For_i(0, loop_iters, 1):
                _loss_body(nc, tc, mm_dtype, npass, fview, aview, tgt, nll,
                           iota_f, singles, feats, psum, stats, scratch)
        else:
            for _rep in range(reps):
                _loss_body(nc, tc, mm_dtype, npass, fview, aview, tgt, nll,
                           iota_f, singles, feats, psum, stats, scratch)

    return nc


def _loss_body(nc, tc, mm_dtype, npass, fview, aview, tgt, nll, iota_f,
               singles, feats, psum, stats, scratch):
    f32 = mybir.dt.float32
    mdt = _mm_dt(mm_dtype)

    if True:
        # anchorT resident in SBUF, loaded per-kt so matmuls can start early
        anchor_sb = singles.tile([P, KT, CLS], mdt, name="anchor_sb")
        for kt in range(KT):
            nc.sync.dma_start(out=anchor_sb[:, kt, :], in_=aview[:, kt, :])

        # per-row target index as f32; column m holds rows [m*128, (m+1)*128)
        tgt_sb = singles.tile([P, MT], f32, name="tgt_sb")
        nc.sync.dma_start(out=tgt_sb, in_=tgt.ap().rearrange("(m p) -> p m", p=P))

        nll_sb = singles.tile([P, MT], f32, name="nll_sb")

        grp = max(1, GRP // npass)  # keep slab SBUF footprint constant
        for g in range(MT // grp):
            # feature slab for grp m-tiles; per-kt DMAs with >=1KB
            # contiguous runs per partition
            slab = feats.tile([P, npass * KT, grp * P], mdt)
            for kt in range(npass * KT):
                nc.sync.dma_start(
                    out=slab[:, kt, :],
                    in_=fview[:, kt, g * grp * P : (g + 1) * grp * P],
                )

            # kt-outer over the group's m-tiles: each arriving anchor/slab
            # chunk unlocks grp*2 matmuls, so PE saturates while the first
            # contraction's data is still streaming in.
            ps_list = [
                psum.tile([P, 2, N0], f32, name="ps", tag="ps")
                for _ in range(grp)
            ]
            for kt in range(npass * KT):
                akt = kt % KT
                for mi in range(grp):
                    msl = slice(mi * P, (mi + 1) * P)
                    nc.tensor.matmul(
                        ps_list[mi][:, 0, :],
                        slab[:, kt, msl],
                        anchor_sb[:, akt, 0:N0],
                        start=(kt == 0),
                        stop=(kt == npass * KT - 1),
                    )
                    nc.tensor.matmul(
                        ps_list[mi][:, 1, 0:N1],
                        slab[:, kt, msl],
                        anchor_sb[:, akt, N0:CLS],
                        start=(kt == 0),
                        stop=(kt == npass * KT - 1),
                    )

            for mi in range(grp):
                m = g * grp + mi
                ps = ps_list[mi]
                # pad unused tail of bank 1 so flat reductions are safe
                nc.vector.memset(ps[:, 1, N1:N0], NEG_BIG)

                flat = ps.rearrange("p a b -> p (a b)")  # [128, 1024]

                nmx = stats.tile([P, 1], f32)  # -max(scores) per row
                nc.vector.tensor_reduce(
                    out=nmx,
                    in_=flat,
                    axis=mybir.AxisListType.X,
                    op=mybir.AluOpType.max,
                    negate=True,
                )

                # exp(scores - max) with fused per-row sum on the ACT engine
                expt = scratch.tile([P, NF], f32, name="expt")
                sume = stats.tile([P, 1], f32)
                nc.scalar.activation(
                    out=expt,
                    in_=flat,
                    func=mybir.ActivationFunctionType.Exp,
                    bias=nmx,
                    scale=1.0,
                    accum_out=sume,
                )

                # s_target = sum_c scores[c] * (iota[c] == target), one DVE pass
                st = stats.tile([P, 1], f32)
                junk = scratch.tile([P, CLS], f32, name="junk")
                nc.vector.scalar_tensor_tensor(
                    out=junk,
                    in0=iota_f,
                    scalar=tgt_sb[:, m : m + 1],
                    in1=flat[:, 0:CLS],
                    op0=mybir.AluOpType.is_equal,
                    op1=mybir.AluOpType.mult,
                    accum_out=st,
                )

                lsum = stats.tile([P, 1], f32)
                nc.scalar.activation(
                    out=lsum, in_=sume, func=mybir.ActivationFunctionType.Ln
                )

                # nll = (log(sum) - (-max)) - s_target = lse - s_target
                nc.vector.scalar_tensor_tensor(
                    out=nll_sb[:, m : m + 1],
                    in0=lsum,
                    scalar=nmx,
                    in1=st,
                    op0=mybir.AluOpType.subtract,
                    op1=mybir.AluOpType.subtract,
                )

        nc.sync.dma_start(out=nll.ap().rearrange("(m p) -> p m", p=P), in_=nll_sb)


def build_program_devT(mm_dtype: str = MM_DTYPE) -> bass.Bass:
    """Variant that takes feature in natural [BPC, FEAT] layout and transposes
    128x128 tiles on the PE (transpose-mode matmul via identity), so no host
    transpose of feature is needed. Anchor still arrives transposed."""
    from concourse.masks import make_identity

    f32 = mybir.dt.float32
    mdt = _mm_dt(mm_dtype)
    assert mm_dtype != "f32r2", "devT variant: single-pass dtypes only"

    nc = bacc.Bacc(None, target_bir_lowering=False, debug=False)
    feat = nc.dram_tensor("feat", [BPC, FEAT], mdt, kind="ExternalInput")
    anchorT = nc.dram_tensor("anchorT", [FEAT, CLS], mdt, kind="ExternalInput")
    tgt = nc.dram_tensor("tgt", [BPC], f32, kind="ExternalInput")
    nll = nc.dram_tensor("nll", [BPC], f32, kind="ExternalOutput")

    fview = feat.ap().rearrange("(mt p) k -> p mt k", p=P)    # [128, 16, 2048]
    aview = anchorT.ap().rearrange("(kt p) c -> p kt c", p=P)  # [128, 16, 1000]

    with tile.TileContext(nc) as tc, ExitStack() as ctx:
        singles = ctx.enter_context(tc.tile_pool(name="singles", bufs=1))
        feats = ctx.enter_context(tc.tile_pool(name="feats", bufs=3))
        featsT = ctx.enter_context(tc.tile_pool(name="featsT", bufs=2))
        psum = ctx.enter_context(tc.tile_pool(name="psum", bufs=2, space="PSUM"))
        psumT = ctx.enter_context(tc.tile_pool(name="psumT", bufs=4, space="PSUM"))
        stats = ctx.enter_context(tc.tile_pool(name="stats", bufs=8))
        scratch = ctx.enter_context(tc.tile_pool(name="scratch", bufs=2))

        anchor_sb = singles.tile([P, KT, CLS], mdt)
        for kt in range(KT):
            nc.sync.dma_start(out=anchor_sb[:, kt, :], in_=aview[:, kt, :])

        identity = singles.tile([P, P], mdt)
        make_identity(nc, identity)

        iota_i = singles.tile([P, CLS], mybir.dt.int32)
        nc.gpsimd.iota(iota_i, pattern=[[1, CLS]], base=0, channel_multiplier=0)
        iota_f = singles.tile([P, CLS], f32)
        nc.vector.tensor_copy(out=iota_f, in_=iota_i)

        tgt_sb = singles.tile([P, MT], f32)
        nc.sync.dma_start(out=tgt_sb, in_=tgt.ap().rearrange("(m p) -> p m", p=P))

        nll_sb = singles.tile([P, MT], f32)

        for m in range(MT):
            # natural-layout m-tile: [128 rows, 2048 feat], split into 4 DMAs
            fm = feats.tile([P, FEAT], mdt, name="fm")
            for q in range(4):
                nc.sync.dma_start(
                    out=fm[:, q * (FEAT // 4) : (q + 1) * (FEAT // 4)],
                    in_=fview[:, m, q * (FEAT // 4) : (q + 1) * (FEAT // 4)],
                )

            # transpose 16 [128,128] tiles on PE, collect featT in SBUF
            fmT = featsT.tile([P, KT, P], mdt, name="fmT")
            for kt in range(0, KT, 2):
                # pack two transposes into one PSUM bank-pair tile
                pst = psumT.tile([P, 2, P], mdt, name="pst")
                for j in range(2):
                    nc.tensor.transpose(
                        pst[:, j, :], fm[:, (kt + j) * P : (kt + j + 1) * P], identity
                    )
                nc.vector.tensor_copy(out=fmT[:, kt : kt + 2, :], in_=pst)

            ps = psum.tile([P, 2, N0], f32, name="ps")
            for kt in range(KT):
                nc.tensor.matmul(
                    ps[:, 0, :],
                    fmT[:, kt, :],
                    anchor_sb[:, kt, 0:N0],
                    start=(kt == 0),
                    stop=(kt == KT - 1),
                )
                nc.tensor.matmul(
                    ps[:, 1, 0:N1],
                    fmT[:, kt, :],
                    anchor_sb[:, kt, N0:CLS],
                    start=(kt == 0),
                    stop=(kt == KT - 1),
                )
            nc.vector.memset(ps[:, 1, N1:N0], NEG_BIG)

            flat = ps.rearrange("p a b -> p (a b)")

            nmx = stats.tile([P, 1], f32)
            nc.vector.tensor_reduce(
                out=nmx,
                in_=flat,
                axis=mybir.AxisListType.X,
                op=mybir.AluOpType.max,
                negate=True,
            )
            expt = scratch.tile([P, NF], f32, name="expt")
            sume = stats.tile([P, 1], f32)
            nc.scalar.activation(
                out=expt,
                in_=flat,
                func=mybir.ActivationFunctionType.Exp,
                bias=nmx,
                scale=1.0,
                accum_out=sume,
            )
            st = stats.tile([P, 1], f32)
            junk = scratch.tile([P, CLS], f32, name="junk")
            nc.vector.scalar_tensor_tensor(
                out=junk,
                in0=iota_f,
                scalar=tgt_sb[:, m : m + 1],
                in1=flat[:, 0:CLS],
                op0=mybir.AluOpType.is_equal,
                op1=mybir.AluOpType.mult,
                accum_out=st,
            )
            lsum = stats.tile([P, 1], f32)
            nc.scalar.activation(
                out=lsum, in_=sume, func=mybir.ActivationFunctionType.Ln
            )
            nc.vector.scalar_tensor_tensor(
                out=nll_sb[:, m : m + 1],
                in0=lsum,
                scalar=nmx,
                in1=st,
                op0=mybir.AluOpType.subtract,
                op1=mybir.AluOpType.subtract,
            )

        nc.sync.dma_start(out=nll.ap().rearrange("(m p) -> p m", p=P), in_=nll_sb)

    return nc


def prepare_inputs_devT(feature, anchor, _target, mm_dtype: str = MM_DTYPE):
    npdt = _np_mm(mm_dtype)
    feature = np.asarray(feature, dtype=np.float32)
    anchor = np.asarray(anchor, dtype=np.float32)
    tgt_f = np.asarray(_target).astype(np.float32)
    anchorT = np.ascontiguousarray(anchor.T).astype(npdt)
    in_maps = []
    for c in range(NCORES):
        sl = slice(c * BPC, (c + 1) * BPC)
        in_maps.append(
            {
                "feat": np.ascontiguousarray(feature[sl]).astype(npdt),
                "anchorT": anchorT,
                "tgt": np.ascontiguousarray(tgt_f[sl]),
            }
        )
    return in_maps


def _np_mm(mm_dtype: str):
    if mm_dtype == "bf16":
        import ml_dtypes

        return np.dtype(ml_dtypes.bfloat16)
    return np.dtype(np.float32)


def prepare_inputs(feature, anchor, _target, mm_dtype: str = MM_DTYPE):
    """Host-side sharding + layout prep. Returns per-core input maps."""
    npdt = _np_mm(mm_dtype)
    feature = np.asarray(feature, dtype=np.float32)
    anchor = np.asarray(anchor, dtype=np.float32)
    tgt_f = np.asarray(_target).astype(np.float32)

    assert mm_dtype != "f32r2", "use prepare_inputs_f32r2"
    anchorT = np.ascontiguousarray(anchor.T).astype(npdt)  # [FEAT, CLS]
    in_maps = []
    for c in range(NCORES):
        sl = slice(c * BPC, (c + 1) * BPC)
        featT_c = np.ascontiguousarray(feature[sl].T).astype(npdt)  # [FEAT, BPC]
        in_maps.append(
            {
                "featT": featT_c,
                "anchorT": anchorT,
                "tgt": np.ascontiguousarray(tgt_f[sl]),
            }
        )
    return in_maps


def prepare_inputs_f32r2(feature, anchor, _target):
    """hi/lo split inputs for the two-pass f32r variant."""
    import ml_dtypes

    feature = np.asarray(feature, dtype=np.float32)
    anchor = np.asarray(anchor, dtype=np.float32)
    tgt_f = np.asarray(_target).astype(np.float32)

    anchorT = np.ascontiguousarray(anchor.T).astype(np.float32)  # [FEAT, CLS]
    in_maps = []
    for c in range(NCORES):
        sl = slice(c * BPC, (c + 1) * BPC)
        fT = np.ascontiguousarray(feature[sl].T)  # [FEAT, BPC]
        f_hi = fT.astype(ml_dtypes.bfloat16).astype(np.float32)
        f_lo = fT - f_hi
        featT_c = np.ascontiguousarray(np.concatenate([f_hi, f_lo], axis=0))
        in_maps.append(
            {
                "featT": featT_c,
                "anchorT": anchorT,
                "tgt": np.ascontiguousarray(tgt_f[sl]),
            }
        )
    return in_maps


_PROGRAM_CACHE: dict = {}


def _get_program(mm_dtype: str, reps: int = 1, variant: str = "hostT") -> bass.Bass:
    key = (mm_dtype, reps, variant)
    nc = _PROGRAM_CACHE.get(key)
    if nc is None:
        if variant == "hostT":
            nc = build_program(mm_dtype, reps=reps)
        elif variant.startswith("loop"):
            nc = build_program(mm_dtype, loop_iters=int(variant[4:]))
        else:
            assert reps == 1
            nc = build_program_devT(mm_dtype)
        nc.compile()  # bacc pass pipeline (reg alloc, wait splitting, ...)
        _PROGRAM_CACHE[key] = nc
    return nc


_RUNNER_CACHE: dict = {}


def make_runner(nc: bass.Bass, in_maps):
    """Compile once; return callable that re-executes with device-resident
    inputs (only the tiny donated output zeros are re-created per call)."""
    import jax
    import jax.core
    from jax.experimental.shard_map import shard_map
    from jax.sharding import Mesh, NamedSharding, PartitionSpec

    from concourse import bass2jax, mybir as mb

    bass2jax.install_neuronx_cc_hook()

    partition_name = (
        nc.partition_id_tensor.name if nc.partition_id_tensor else None
    )
    in_names, out_names, out_avals, zero_shapes = [], [], [], []
    for alloc in nc.m.functions[0].allocations:
        if not isinstance(alloc, mb.MemoryLocationSet):
            continue
        name = alloc.memorylocations[0].name
        if alloc.kind == "ExternalInput":
            if name != partition_name:
                in_names.append(name)
        elif alloc.kind == "ExternalOutput":
            shape = tuple(alloc.tensor_shape)
            dtype = mb.dt.np(alloc.dtype)
            out_names.append(name)
            out_avals.append(jax.core.ShapedArray(shape, dtype))
            zero_shapes.append((shape, dtype))
    n_params = len(in_names)
    n_outs = len(out_names)
    all_in_names = list(in_names) + list(out_names)
    if partition_name is not None:
        all_in_names.append(partition_name)

    donate = tuple(range(n_params, n_params + n_outs))

    def _body(*args):
        operands = list(args)
        if partition_name is not None:
            operands.append(bass2jax.partition_id_tensor())
        outs = bass2jax._bass_exec_p.bind(
            *operands,
            out_avals=tuple(out_avals),
            in_names=tuple(all_in_names),
            out_names=tuple(out_names),
            lowering_input_output_aliases=(),
            sim_require_finite=True,
            sim_require_nnan=True,
            nc=nc,
        )
        return tuple(outs)

    devices = jax.devices()[:NCORES]
    mesh = Mesh(np.asarray(devices), ("core",))
    in_specs = (PartitionSpec("core"),) * (n_params + n_outs)
    out_specs = (PartitionSpec("core"),) * n_outs
    sharded = jax.jit(
        shard_map(
            _body, mesh=mesh, in_specs=in_specs, out_specs=out_specs,
            check_rep=False,
        ),
        donate_argnums=donate,
        keep_unused=True,
    )
    sharding = NamedSharding(mesh, PartitionSpec("core"))
    dev_in = [
        jax.device_put(
            np.concatenate([np.asarray(in_maps[c][nm]) for c in range(NCORES)], axis=0),
            sharding,
        )
        for nm in in_names
    ]
    jax.block_until_ready(dev_in)

    def run():
        zeros = [
            np.zeros((NCORES * s[0], *s[1:]), dt) for (s, dt) in zero_shapes
        ]
        outs = sharded(*dev_in, *zeros)
        jax.block_until_ready(outs)
        return {
            nm: np.asarray(outs[i]).reshape(NCORES, *out_avals[i].shape)
            for i, nm in enumerate(out_names)
        }

    return run


def timed_run(in_maps, mm_dtype: str = MM_DTYPE, reps: int = 1, iters: int = 3,
              variant: str = "hostT"):
    """Compile the reps-times-repeated program, return best wall seconds/call."""
    import time

    key = (mm_dtype, reps, variant, id(in_maps))
    runner = _RUNNER_CACHE.get(key)
    if runner is None:
        nc = _get_program(mm_dtype, reps=reps, variant=variant)
        runner = make_runner(nc, in_maps)
        _RUNNER_CACHE[key] = runner
    runner()  # warmup (compile + first exec)
    best = float("inf")
    for _ in range(iters):
        t0 = time.perf_counter()
        runner()
        best = min(best, time.perf_counter() - t0)
    return best


def run_on_cores(in_maps, mm_dtype: str = MM_DTYPE, trace: bool = False):
    from concourse.bass_utils import run_bass_kernel_spmd

    nc = _get_program(mm_dtype)
    res = run_bass_kernel_spmd(nc, in_maps, list(range(NCORES)), trace=trace)
    return res


def kernel(feature, anchor, _target) -> np.ndarray:
    mm_dtype = MM_DTYPE
    if mm_dtype == "f32r2":
        in_maps = prepare_inputs_f32r2(feature, anchor, _target)
    else:
        in_maps = prepare_inputs(feature, anchor, _target, mm_dtype)
    res = run_on_cores(in_maps, mm_dtype, trace=os.environ.get("CL_TRACE", "") == "1")
    nll_all = np.concatenate([res.results[c]["nll"] for c in range(NCORES)])
    if os.environ.get("CL_TRACE", "") == "1" and res.exec_time_ns is not None:
        print(f"HW exec time: {res.exec_time_ns} ns")
    return np.asarray(np.mean(nll_all, dtype=np.float64), dtype=np.float32)

